# revision 3
# baseline (speedup 1.0000x reference)
"""Trainium2 Bass kernel for the Clebsch-Gordan tensor product
(nn_CG_woFilter_cuda, LMAX=5, tau=16, B=256).

Strategy (8 cores, data-parallel over batch, 32 b/core):
  out[b,i,j,m] = sum_{p,q} C[p,q,m] * F1[b,i,p] * F2[b,j,q]   (complex)
is evaluated as a block-diagonal matmul over stacked (pair,p,q) rows:
  - activations are pre-transposed on host to AT[(l,p), (b,i)] (re/im),
  - per row-group g (packed (l1,l2) pairs, K_g<=128): replicate AT rows
    into F1/F2 row layouts with one-hot matmuls on TensorE,
  - 4 outer products rr/ri/ir/ii as broadcast tensor_tensor on DVE/GpSimd
    over free dim (b,i,j), written as float32r,
  - out_re/out_im = W+ @ rr + W- @ ii / W+ @ (ri+ir) via PSUM-accumulated
    fp32r matmuls (W block-diagonal per group),
  - ScalarE evacuates PSUM -> SBUF, DMA to a device-layout output
    [2, 457, 32*256]; host reassembles the reference layout.
"""

import math

import numpy as np

# ---------------------------------------------------------------- constants
LMAX = 5
TAU = 16
BATCH = 256
NCORES = 8
BPC = BATCH // NCORES
DEG = [2 * l + 1 for l in range(LMAX + 1)]
CUM = np.concatenate([[0], (TAU * (2 * np.arange(LMAX + 1) + 1)).cumsum()]).astype(int)
LPOFF = np.concatenate([[0], np.cumsum(DEG)]).astype(int)
NROW_AT = int(LPOFF[-1])                       # 36
FREE = BPC * TAU * TAU                         # 8192 free elements per core
NB_CHUNK = 8                                   # batches per product chunk
CHUNK = NB_CHUNK * TAU * TAU                   # 2048
NCHUNK = BPC // NB_CHUNK                       # 4
SUB = 1024                                     # psum/evac granularity


def _cg(l1, m1, l2, m2, l, m):
    if m1 + m2 != m:
        return 0.0
    f = math.factorial
    pre = math.sqrt((2 * l + 1) * f(l + l1 - l2) * f(l - l1 + l2)
                    * f(l1 + l2 - l) / f(l1 + l2 + l + 1))
    pre *= math.sqrt(f(l + m) * f(l - m) * f(l1 - m1) * f(l1 + m1)
                     * f(l2 - m2) * f(l2 + m2))
    kmin = max(0, l2 - l - m1, l1 + m2 - l)
    kmax = min(l1 + l2 - l, l1 - m1, l2 + m2)
    s = 0.0
    for k in range(kmin, kmax + 1):
        s += (-1) ** k / (f(k) * f(l1 + l2 - l - k) * f(l1 - m1 - k)
                          * f(l2 + m2 - k) * f(l - l2 + m1 + k) * f(l - l1 - m2 + k))
    return pre * s


TRIPLES = []
for l in range(LMAX + 1):
    for l1 in range(LMAX + 1):
        for l2 in range(l1 + 1):
            if abs(l1 - l2) <= l <= l1 + l2:
                TRIPLES.append((l, l1, l2))

CG = {}
for (l, l1, l2) in TRIPLES:
    C = np.zeros((2 * l1 + 1, 2 * l2 + 1, 2 * l + 1), np.float64)
    for m1 in range(-l1, l1 + 1):
        for m2 in range(-l2, l2 + 1):
            m = m1 + m2
            if -l <= m <= l:
                C[m1 + l1, m2 + l2, m + l] = _cg(l1, m1, l2, m2, l, m)
    CG[(l, l1, l2)] = C

TRIPLE_OFF = {}
_off = 0
for _t in TRIPLES:
    TRIPLE_OFF[_t] = _off
    _off += TAU * TAU * (2 * _t[0] + 1)
OUT_DIM1 = _off                                # 116992

GROUP_PAIRS = [
    [(5, 5), (3, 0)],
    [(5, 4), (2, 2), (1, 0)],
    [(4, 4), (4, 2), (0, 0)],
    [(5, 3), (3, 3)],
    [(4, 3), (5, 2), (1, 1)],
    [(3, 2), (5, 1), (4, 1), (3, 1), (5, 0)],
    [(2, 1), (4, 0), (2, 0)],
]
NG = len(GROUP_PAIRS)

GROUPS = []
_orb = 0
for _pairs in GROUP_PAIRS:
    rows, cols = [], []
    for (l1, l2) in _pairs:
        ls = [l for l in range(LMAX + 1) if abs(l1 - l2) <= l <= l1 + l2]
        for p in range(2 * l1 + 1):
            for q in range(2 * l2 + 1):
                rows.append((l1, l2, p, q))
        for l in ls:
            for m in range(2 * l + 1):
                cols.append((l, l1, l2, m))
    K, M = len(rows), len(cols)
    W = np.zeros((K, M), np.float64)
    for r, (l1, l2, p, q) in enumerate(rows):
        for c, (l, cl1, cl2, m) in enumerate(cols):
            if (cl1, cl2) == (l1, l2):
                W[r, c] = CG[(l, l1, l2)][p, q, m]
    R1 = np.zeros((NROW_AT, K), np.float32)
    R2 = np.zeros((NROW_AT, K), np.float32)
    for r, (l1, l2, p, q) in enumerate(rows):
        R1[LPOFF[l1] + p, r] = 1.0
        R2[LPOFF[l2] + q, r] = 1.0
    GROUPS.append(dict(pairs=_pairs, cols=cols, K=K, M=M,
                       W=W.astype(np.float32), R1=R1, R2=R2, out_row_base=_orb))
    _orb += M
NOUTROW = _orb                                 # 457

ROW_OF = {}
for _G in GROUPS:
    for _c, (_l, _l1, _l2, _m) in enumerate(_G["cols"]):
        ROW_OF[(_l, _l1, _l2, _m)] = _G["out_row_base"] + _c


# ------------------------------------------------------- BIR wait-splitting
def _fix_bir_waits(bir_bytes, max_waits=1):
    """walrus on this toolchain rejects >1 embedded sync wait per
    instruction; hoist excess waits onto preceding same-engine NoOps."""
    import json
    bir = json.loads(bir_bytes)
    ctr = 0
    for f in bir["functions"]:
        for blk in f["blocks"]:
            out = []
            for ins in blk["instructions"]:
                si = ins.get("sync_info")
                waits = (si or {}).get("on_wait") or []
                if len(waits) > max_waits:
                    excess, keep = waits[:-max_waits], waits[-max_waits:]
                    si["on_wait"] = keep
                    for ci in range(0, len(excess), max_waits):
                        ctr += 1
                        out.append({
                            "debug": ins.get("debug", 0),
                            "engine": ins["engine"], "ins": [], "outs": [],
                            "name": f"I-waitfix-{ctr}", "opcode": "NoOp",
                            "sync_info": {"on_update": [],
                                          "on_wait": excess[ci:ci + max_waits]},
                        })
                out.append(ins)
            blk["instructions"] = out
    return json.dumps(bir).encode()


# ------------------------------------------------------------ device program
def build_nc(ii_engine="gpsimd"):
    import concourse.bass as bass
    import concourse.mybir as mybir
    from concourse.tile import TileContext

    f32 = mybir.dt.float32
    f32r = mybir.dt.float32r
    MUL = mybir.AluOpType.mult

    nc = bass.Bass()
    atre = nc.dram_tensor("atre", [NROW_AT, BPC * TAU], f32r, kind="ExternalInput")
    atim = nc.dram_tensor("atim", [NROW_AT, BPC * TAU], f32r, kind="ExternalInput")
    wpos = nc.dram_tensor("wpos", [128, NG * 128], f32r, kind="ExternalInput")
    wneg = nc.dram_tensor("wneg", [128, NG * 128], f32r, kind="ExternalInput")
    r1 = nc.dram_tensor("r1", [NROW_AT, NG * 128], f32r, kind="ExternalInput")
    r2 = nc.dram_tensor("r2", [NROW_AT, NG * 128], f32r, kind="ExternalInput")
    out = nc.dram_tensor("out", [2, NOUTROW, FREE], f32, kind="ExternalOutput")

    with TileContext(nc) as tc:
        with tc.tile_pool(name="const", bufs=1) as cpool, \
             tc.tile_pool(name="rep", bufs=2) as rpool, \
             tc.tile_pool(name="pp", bufs=2) as ppool, \
             tc.tile_pool(name="stage", bufs=3) as spool, \
             tc.tile_pool(name="ps", bufs=2, space="PSUM") as psum:

            atre_t = cpool.tile([NROW_AT, BPC * TAU], f32r)
            atim_t = cpool.tile([NROW_AT, BPC * TAU], f32r)
            wpos_t = cpool.tile([128, NG * 128], f32r)
            wneg_t = cpool.tile([128, NG * 128], f32r)
            r1_t = cpool.tile([NROW_AT, NG * 128], f32r)
            r2_t = cpool.tile([NROW_AT, NG * 128], f32r)
            for t, d in [(atre_t, atre), (atim_t, atim), (wpos_t, wpos),
                         (wneg_t, wneg), (r1_t, r1), (r2_t, r2)]:
                nc.sync.dma_start(out=t[:], in_=d[:])

            for g, G in enumerate(GROUPS):
                Kg, Mg = G["K"], G["M"]
                rb = G["out_row_base"]
                gs = slice(g * 128, g * 128 + Kg)
                wp = wpos_t[:Kg, g * 128:g * 128 + Mg]
                wn = wneg_t[:Kg, g * 128:g * 128 + Mg]

                reps = {}
                for nm, rt, at in [("f1r", r1_t, atre_t), ("f1i", r1_t, atim_t),
                                   ("f2r", r2_t, atre_t), ("f2i", r2_t, atim_t)]:
                    ps = psum.tile([128, SUB], f32, tag="pre")
                    nc.tensor.matmul(ps[:Kg, 0:BPC * TAU], rt[:, gs], at[:],
                                     start=True, stop=True)
                    st = rpool.tile([128, BPC * TAU], f32, tag=nm)
                    nc.scalar.copy(out=st[:Kg, :], in_=ps[:Kg, 0:BPC * TAU])
                    reps[nm] = st

                for ch in range(NCHUNK):
                    cs = slice(ch * NB_CHUNK * TAU, (ch + 1) * NB_CHUNK * TAU)

                    def f1ap(t):
                        return (t[:Kg, cs]
                                .rearrange("k (b i) -> k b i", b=NB_CHUNK)
                                .unsqueeze(3)
                                .broadcast_to([Kg, NB_CHUNK, TAU, TAU]))

                    def f2ap(t):
                        return (t[:Kg, cs]
                                .rearrange("k (b j) -> k b j", b=NB_CHUNK)
                                .unsqueeze(2)
                                .broadcast_to([Kg, NB_CHUNK, TAU, TAU]))

                    prods = {}
                    for nm, a, b, eng in [
                            ("rr", "f1r", "f2r", nc.vector),
                            ("ri", "f1r", "f2i", nc.vector),
                            ("ir", "f1i", "f2r", nc.vector),
                            ("ii", "f1i", "f2i",
                             nc.gpsimd if ii_engine == "gpsimd" else nc.vector)]:
                        pt = ppool.tile([128, CHUNK], f32r, tag=nm)
                        oap = pt[:Kg, :].rearrange("k (b i j) -> k b i j",
                                                   b=NB_CHUNK, i=TAU)
                        eng.tensor_tensor(out=oap, in0=f1ap(reps[a]),
                                          in1=f2ap(reps[b]), op=MUL)
                        prods[nm] = pt

                    for sb in range(CHUNK // SUB):
                        pre = psum.tile([128, SUB], f32, tag="pre")
                        pim = psum.tile([128, SUB], f32, tag="pim")
                        for ss in range(SUB // 512):
                            psl = slice(ss * 512, (ss + 1) * 512)
                            csl = slice(sb * SUB + ss * 512,
                                        sb * SUB + (ss + 1) * 512)
                            nc.tensor.matmul(pre[:Mg, psl], wp,
                                             prods["rr"][:Kg, csl],
                                             start=True, stop=False)
                            nc.tensor.matmul(pim[:Mg, psl], wp,
                                             prods["ri"][:Kg, csl],
                                             start=True, stop=False)
                            nc.tensor.matmul(pim[:Mg, psl], wp,
                                             prods["ir"][:Kg, csl],
                                             start=False, stop=True)
                            nc.tensor.matmul(pre[:Mg, psl], wn,
                                             prods["ii"][:Kg, csl],
                                             start=False, stop=True)
                        off = ch * CHUNK + sb * SUB
                        for c, pt, nm in [(0, pre, "stre"), (1, pim, "stim")]:
                            st = spool.tile([128, SUB], f32, tag=nm)
                            nc.scalar.copy(out=st[:Mg, :], in_=pt[:Mg, :])
                            nc.sync.dma_start(
                                out=out[c, rb:rb + Mg, off:off + SUB],
                                in_=st[:Mg, :])
    nc.to_json_bytes_orig = nc.to_json_bytes
    nc.to_json_bytes = lambda: _fix_bir_waits(nc.to_json_bytes_orig())
    return nc


# ---------------------------------------------------------------- host side
def _make_at(act_slice):
    B = act_slice.shape[0]
    atre = np.zeros((NROW_AT, B * TAU), np.float32)
    atim = np.zeros((NROW_AT, B * TAU), np.float32)
    for l in range(LMAX + 1):
        d = 2 * l + 1
        blk = act_slice[:, CUM[l]:CUM[l + 1], :].reshape(B, TAU, d, 2)
        atre[LPOFF[l]:LPOFF[l] + d] = blk[..., 0].transpose(2, 0, 1).reshape(d, -1)
        atim[LPOFF[l]:LPOFF[l] + d] = blk[..., 1].transpose(2, 0, 1).reshape(d, -1)
    return atre, atim


def _consts():
    wpos = np.zeros((128, NG * 128), np.float32)
    wneg = np.zeros((128, NG * 128), np.float32)
    r1 = np.zeros((NROW_AT, NG * 128), np.float32)
    r2 = np.zeros((NROW_AT, NG * 128), np.float32)
    for g, G in enumerate(GROUPS):
        K, M = G["K"], G["M"]
        wpos[:K, g * 128:g * 128 + M] = G["W"]
        wneg[:K, g * 128:g * 128 + M] = -G["W"]
        r1[:, g * 128:g * 128 + K] = G["R1"]
        r2[:, g * 128:g * 128 + K] = G["R2"]
    return wpos, wneg, r1, r2


def _assemble(dev_outs):
    out = np.empty((BATCH, OUT_DIM1, 2), np.float32)
    for core, dv in enumerate(dev_outs):
        b0 = core * BPC
        dvv = dv.reshape(2, NOUTROW, BPC, TAU, TAU)
        for t in TRIPLES:
            l = t[0]
            d = 2 * l + 1
            r0 = ROW_OF[(l, t[1], t[2], 0)]
            blk = dvv[:, r0:r0 + d].transpose(2, 3, 4, 1, 0)
            out[b0:b0 + BPC, TRIPLE_OFF[t]:TRIPLE_OFF[t] + TAU * TAU * d] = \
                blk.reshape(BPC, TAU * TAU * d, 2)
    return out


_NC_CACHE = {}


def kernel(activations: np.ndarray, _trace: bool = False):
    import os
    from concourse.bass_utils import run_bass_kernel_spmd

    act = np.ascontiguousarray(np.asarray(activations), dtype=np.float32)
    assert act.shape == (BATCH, int(CUM[-1]), 2), act.shape

    if "nc" not in _NC_CACHE:
        _NC_CACHE["nc"] = build_nc()
    nc = _NC_CACHE["nc"]

    wpos, wneg, r1, r2 = _consts()
    in_maps = []
    for core in range(NCORES):
        atre, atim = _make_at(act[core * BPC:(core + 1) * BPC])
        in_maps.append(dict(atre=atre, atim=atim, wpos=wpos, wneg=wneg,
                            r1=r1, r2=r2))
    kwargs = {}
    if _trace or os.environ.get("KERNEL_TRACE"):
        kwargs = dict(trace=True, trace_cores=[0])
    res = run_bass_kernel_spmd(nc, in_maps, core_ids=list(range(NCORES)),
                               **kwargs)
    _NC_CACHE["last_result"] = res
    return _assemble([r["out"] for r in res.results])
